# revision 1
# baseline (speedup 1.0000x reference)
"""Beamformer (MoE-style per-frame beam dispatch) for Trainium2, 8 NeuronCores.

Math per frame n (w = W[beam_id[n]]):
    out_r[n,f] = sum_c xr*wr + xi*wi
    out_i[n,f] = sum_c xi*wr - xr*wi          -> out (16384, 2, 257, 1) fp32

Strategy (all fp32, exact):
  * Frames are globally sorted by beam on the host, so each beam occupies one
    contiguous span of the frame axis. The per-frame weight gather then
    becomes a handful of per-beam segments -- no on-device gather at all.
  * Shard the 257 frequency bins: core c owns bins [32c, 32c+32) as 4
    "bingroups" of 8 bins; bin 256 is done on host (1/257 of the work) so the
    SPMD program is identical on all 8 cores. Every core streams all frames.
  * The complex filter-and-sum is a matmul on the tensor engine: contraction
    dim K = 8 bins x 16 (re/im x 8 channels), stationary operand = a
    block-diagonal weight tile [128, 32] per (beam, bingroup) (16 real
    outputs: 8 bins x re/im), moving operand = transposed input columns
    (frames). 4 bingroups are packed into the 128 PSUM partitions with
    col-tiled matmuls (tile_position), so PSUM->SBUF evacuation and the
    output DMA run at full 128-partition width.
  * Per 4096-frame chunk: one x DMA [128, 4, 4096], per beam-segment piece
    (<=512 cols) 4 matmuls -> PSUM, DVE/ACT (alternating) copy -> staging,
    one [128, 4096] DMA out. DMA per core ~42 MB, which is the roofline.

Host side: one global transpose/pack of the sorted input (~1 s), per-core
inputs are contiguous slices of it; output is un-permuted at the end.
The Bass program depends only on the beam histogram (segment boundaries are
baked in as static sizes); it is built and compiled on first call.
"""

import numpy as np

NUM_BEAM, NUM_BIN, NUM_CH = 24, 257, 8
N_FRAMES = 16384
NCORES = 8
P = 128
NBIN_DEV = 256                # bins computed on device
NGRP = NBIN_DEV // 8          # 32 bingroups
GPC = NGRP // NCORES          # 4 bingroups per core
NTW = NUM_BEAM * GPC          # 96 weight tiles per core
FCH = 4096                    # frames per chunk
NCH = N_FRAMES // FCH         # 4 chunks
MAXN = 512                    # max matmul moving dim (one PSUM bank, fp32)

_CACHE = {}
TRACE = False
LAST_RESULTS = None


def _segments(offs):
    """Static per-chunk list of (beam, lo, hi) pieces (local cols, <=MAXN)."""
    chunks = []
    for q in range(NCH):
        n0, n1 = q * FCH, (q + 1) * FCH
        segs = []
        for b in range(NUM_BEAM):
            s0, s1 = max(offs[b], n0), min(offs[b + 1], n1)
            if s1 <= s0:
                continue
            L = s1 - s0
            npieces = -(-L // MAXN)
            bounds = [s0 + (L * i) // npieces for i in range(npieces + 1)]
            for i in range(npieces):
                segs.append((b, bounds[i] - n0, bounds[i + 1] - n0))
        chunks.append(segs)
    return chunks


def _build_program(offs):
    import concourse.bacc as bacc
    import concourse.bass as bass
    import concourse.tile as tile
    from concourse import mybir

    f32 = mybir.dt.float32
    chunks = _segments(offs)

    nc = bacc.Bacc("TRN2", target_bir_lowering=False, debug=False)
    xt_d = nc.dram_tensor("xt", [GPC, P, N_FRAMES], f32, kind="ExternalInput")
    wt_d = nc.dram_tensor("wt", [8, 16, NTW, 2], f32, kind="ExternalInput")
    out_d = nc.dram_tensor("out", [NCH, P, FCH], f32, kind="ExternalOutput")

    with tile.TileContext(nc) as tc:
        with (
            tc.tile_pool(name="singles", bufs=1) as singles,
            tc.tile_pool(name="xp", bufs=2) as xp,
            tc.tile_pool(name="st", bufs=2) as stp,
            tc.tile_pool(name="ps", bufs=8, space=bass.MemorySpace.PSUM) as ps,
        ):
            # block-diagonal weight bank [128, NTW, 32]:
            # w_bank[fs*16+k, b*GPC+g_loc, fs*2+ri'] = W16[ri'][b, bin, k]
            w_bank = singles.tile([P, NTW, 32], f32)
            nc.vector.memset(w_bank[:], 0.0)
            for fs in range(8):
                nc.sync.dma_start(
                    out=w_bank[fs * 16 : (fs + 1) * 16, :, fs * 2 : fs * 2 + 2],
                    in_=wt_d[fs],
                )

            ncopy = 0
            for q in range(NCH):
                x_sb = xp.tile([P, GPC, FCH], f32, tag="x")
                nc.sync.dma_start(
                    out=x_sb[:],
                    in_=xt_d[:, :, q * FCH : (q + 1) * FCH].rearrange(
                        "g p n -> p g n"
                    ),
                )
                st = stp.tile([P, FCH], f32, tag="st")
                for b, lo, hi in chunks[q]:
                    pl = hi - lo
                    acc = ps.tile([P, MAXN], f32, tag="acc")
                    for j in range(GPC):
                        nc.tensor.matmul(
                            acc[32 * j : 32 * j + 32, :pl],
                            w_bank[:, b * GPC + j, :],
                            x_sb[:, j, lo:hi],
                            start=True,
                            stop=True,
                            tile_position=(0, 32 * j),
                        )
                    if ncopy % 2 == 0:
                        nc.vector.tensor_copy(st[:, lo:hi], acc[:, :pl])
                    else:
                        nc.scalar.copy(out=st[:, lo:hi], in_=acc[:, :pl])
                    ncopy += 1
                nc.sync.dma_start(out=out_d[q], in_=st[:])

    nc.compile()
    return nc


def _pack_weights(W):
    """Per-core compact weight tables, each (8, 16, NTW, 2) fp32."""
    wr = W[:, 0]  # (24, 257, 8)
    wi = W[:, 1]
    w16 = np.zeros((NUM_BEAM, NGRP, 8, 16, 2), np.float32)  # b, g, fs, k, ri'
    for g in range(NGRP):
        for fs in range(8):
            fb = g * 8 + fs
            w16[:, g, fs, 0:8, 0] = wr[:, fb]
            w16[:, g, fs, 8:16, 0] = wi[:, fb]
            w16[:, g, fs, 0:8, 1] = -wi[:, fb]
            w16[:, g, fs, 8:16, 1] = wr[:, fb]
    out = []
    for c in range(NCORES):
        sl = w16[:, c * GPC : (c + 1) * GPC]  # (24, GPC, 8, 16, 2)
        out.append(
            np.ascontiguousarray(sl.transpose(2, 3, 0, 1, 4).reshape(8, 16, NTW, 2))
        )
    return out


def _pack_x_global(inp, perm):
    """x_t (NGRP, 128, N): [g, fs*16+ri*8+c, n] = inp[perm[n], ri, 8g+fs, c]."""
    xs = inp[perm][:, :, :NBIN_DEV, :]  # (N, 2, 256, 8)
    arr = xs.reshape(N_FRAMES, 2, NGRP, 8, NUM_CH).transpose(2, 3, 1, 4, 0)
    return np.ascontiguousarray(arr.reshape(NGRP, P, N_FRAMES))


def kernel(**inputs):
    global LAST_RESULTS
    from concourse.bass_utils import run_bass_kernel_spmd

    inp = np.ascontiguousarray(np.asarray(inputs["input"], dtype=np.float32))
    W = np.ascontiguousarray(np.asarray(inputs["W"], dtype=np.float32))
    bid = np.asarray(inputs["beam_id"]).astype(np.int64)

    perm = np.argsort(bid, kind="stable")
    counts = np.bincount(bid, minlength=NUM_BEAM)
    offs = np.concatenate([[0], np.cumsum(counts)]).astype(int)

    key = tuple(offs)
    if key not in _CACHE:
        _CACHE[key] = _build_program(offs)
    nc = _CACHE[key]

    wts = _pack_weights(W)
    xt = _pack_x_global(inp, perm)
    in_maps = [
        {"xt": xt[c * GPC : (c + 1) * GPC], "wt": wts[c]} for c in range(NCORES)
    ]

    res = run_bass_kernel_spmd(nc, in_maps, list(range(NCORES)), trace=TRACE)
    LAST_RESULTS = res

    # device rows (j, m) at partition 32j+m, m = fs*2+ri (m < 16 real)
    out_sorted = np.empty((N_FRAMES, 2, NUM_BIN), np.float32)
    for c in range(NCORES):
        ot = res.results[c]["out"]  # (NCH, 128, FCH)
        a = ot.reshape(NCH, GPC, 32, FCH)[:, :, :16, :]
        a = a.reshape(NCH, GPC, 8, 2, FCH).transpose(0, 4, 3, 1, 2)
        out_sorted[:, :, 32 * c : 32 * c + 32] = a.reshape(N_FRAMES, 2, 32)

    # bin 256 on host (keeps the device bin count divisible by 8 cores)
    xs = inp[:, :, NUM_BIN - 1, :]
    ws = W[bid][:, :, NUM_BIN - 1, :]
    xr, xi = xs[:, 0], xs[:, 1]
    wr, wi = ws[:, 0], ws[:, 1]

    out_full = np.empty((N_FRAMES, 2, NUM_BIN), np.float32)
    out_full[perm] = out_sorted
    out_full[:, 0, NUM_BIN - 1] = (xr * wr + xi * wi).sum(-1)
    out_full[:, 1, NUM_BIN - 1] = (xi * wr - xr * wi).sum(-1)
    return out_full.reshape(N_FRAMES, 2, NUM_BIN, 1)



# revision 3
# speedup vs baseline: 1.0361x; 1.0361x over previous
"""Beamformer (MoE-style per-frame beam dispatch) for Trainium2, 8 NeuronCores.

Math per frame n (w = W[beam_id[n]]):
    out_r[n,f] = sum_c xr*wr + xi*wi
    out_i[n,f] = sum_c xi*wr - xr*wi          -> out (16384, 2, 257, 1) fp32

Strategy (bf16 wire format; gate is rel_err < 2e-2, bf16 lands ~4e-3):
  * Frames globally sorted by beam on the host -> per-beam contiguous spans;
    the per-frame weight gather becomes static per-beam segments. Bins are
    sharded: core c owns bins [32c, 32c+32) as 4 bingroups of 8 bins; bin 256
    is done on host (1/257 of the work, keeps the SPMD program identical).
  * All DMA payloads are bf16 with one contiguous run per partition per
    transfer (descriptor-efficient): x is packed per chunk as [128, GPC*fch],
    the block-diagonal weight bank ships prebuilt as [128, NTW, 32].
  * Matmul: contraction K=128 (8 bins x re/im x 8 ch), stationary [128, 32]
    per (beam, bingroup). Bingroup pairs share one 32-col PE tile position
    (even bingroup -> stationary cols 0-15, odd -> 16-31, zeros elsewhere)
    accumulated via start/stop into the same PSUM rows -> 64 PSUM partitions
    all real. PSUM->SBUF copies (cast to bf16) alternate DVE/ACT.
  * Uneven chunks (3904x3, 3200, 1024, 448 frames): big early chunks give
    ~31KB DMA descriptors (one per partition per chunk, near wire speed);
    the small tail shrinks the pipeline drain after the last x transfer
    (last-chunk compute + final out DMA). Output staged [128, sum(fch)/2]
    (half-chunks stacked on the partition axis) and written per chunk-group
    so out DMAs overlap compute.
"""

import numpy as np

NUM_BEAM, NUM_BIN, NUM_CH = 24, 257, 8
N_FRAMES = 16384
NCORES = 8
P = 128
NBIN_DEV = 256                # bins computed on device
NGRP = NBIN_DEV // 8          # 32 bingroups of 8 bins
GPC = NGRP // NCORES          # 4 bingroups per core
NTW = NUM_BEAM * GPC          # 96 weight tiles per core
CH_SIZES = [3904, 3904, 3904, 3200, 1024, 448]  # frames per chunk (sum = 16384)
NCH = len(CH_SIZES)
F0 = np.concatenate([[0], np.cumsum(CH_SIZES)]).astype(int)   # frame offsets
OC = (F0 // 2).astype(int)    # output column offsets (half-chunk width)
GROUPS = [(0, 2), (2, 4), (4, 6)]     # chunk ranges per out staging/DMA
MAXN = 512                    # max matmul moving dim (one PSUM bank, fp32)

_CACHE = {}
TRACE = False
LAST_RESULTS = None


def _segments(offs):
    """Per (chunk, half): list of (beam, lo, hi) pieces (local cols, <=MAXN)."""
    out = []
    for q in range(NCH):
        hf = CH_SIZES[q] // 2
        for half in range(2):
            n0 = F0[q] + half * hf
            n1 = n0 + hf
            segs = []
            for b in range(NUM_BEAM):
                s0, s1 = max(offs[b], n0), min(offs[b + 1], n1)
                if s1 <= s0:
                    continue
                L = s1 - s0
                npieces = -(-L // MAXN)
                bounds = [s0 + (L * i) // npieces for i in range(npieces + 1)]
                for i in range(npieces):
                    segs.append((b, bounds[i] - n0, bounds[i + 1] - n0))
            out.append(segs)
    return out


def _build_program(offs):
    import concourse.bacc as bacc
    import concourse.bass as bass
    import concourse.tile as tile
    from concourse import mybir

    f32 = mybir.dt.float32
    bf16 = mybir.dt.bfloat16
    halves = _segments(offs)

    nc = bacc.Bacc("TRN2", target_bir_lowering=False, debug=False)
    xt_d = nc.dram_tensor("xt", [P, GPC * N_FRAMES], bf16, kind="ExternalInput")
    wt_d = nc.dram_tensor("wt", [P, NTW, 32], bf16, kind="ExternalInput")
    out_d = nc.dram_tensor("out", [P, N_FRAMES // 2], bf16, kind="ExternalOutput")

    with tile.TileContext(nc) as tc:
        with (
            tc.tile_pool(name="singles", bufs=1) as singles,
            tc.tile_pool(name="xp", bufs=3) as xp,
            tc.tile_pool(name="st", bufs=2) as stp,
            tc.tile_pool(name="ps", bufs=8, space=bass.MemorySpace.PSUM) as ps,
        ):
            w_bank = singles.tile([P, NTW, 32], bf16)
            nc.scalar.dma_start(out=w_bank[:], in_=wt_d[:])

            g_of = {}
            for (g0, g1) in GROUPS:
                for q in range(g0, g1):
                    g_of[q] = (g0, g1)

            ncopy = 0
            st = None
            for q in range(NCH):
                fch = CH_SIZES[q]
                hf = fch // 2
                x_sb = xp.tile([P, GPC, fch], bf16, tag="x")
                nc.sync.dma_start(
                    out=x_sb[:],
                    in_=xt_d[:, GPC * F0[q] : GPC * F0[q + 1]],
                )
                g0, g1 = g_of[q]
                if q == g0:
                    st = stp.tile([P, OC[g1] - OC[g0]], bf16, tag="st")
                for half in range(2):
                    for b, lo, hi in halves[2 * q + half]:
                        pl = hi - lo
                        acc = ps.tile([P, MAXN], f32, tag="acc")
                        for j in range(GPC):
                            h = j // 2
                            nc.tensor.matmul(
                                acc[32 * h : 32 * h + 32, :pl],
                                w_bank[:, b * GPC + j, :],
                                x_sb[:, j, half * hf + lo : half * hf + hi],
                                start=(j % 2 == 0),
                                stop=(j % 2 == 1),
                                tile_position=(0, 32 * h),
                            )
                        co = OC[q] - OC[g0]
                        dst = st[64 * half : 64 * half + 64, co + lo : co + hi]
                        if ncopy % 2 == 0:
                            nc.vector.tensor_copy(dst, acc[:64, :pl])
                        else:
                            nc.scalar.copy(out=dst, in_=acc[:64, :pl])
                        ncopy += 1
                if q == g1 - 1:
                    nc.scalar.dma_start(
                        out=out_d[:, OC[g0] : OC[g1]], in_=st[:]
                    )

    nc.compile()
    return nc


def _pack_weights(W):
    """Per-core block-diagonal stationary banks, each (128, NTW, 32) bf16.

    Row p = fs*16 + (ri*8 + ch); tile tw = b*GPC + j; col = (j%2)*16 + fs*2 + ri'
    holds the conjugate filter-and-sum coefficients:
      ri'=0: [wr | wi],  ri'=1: [-wi | wr]  (stacked over ch in the row dim).
    """
    import ml_dtypes

    wr = W[:, 0]  # (24, 257, 8)
    wi = W[:, 1]
    w16 = np.zeros((NUM_BEAM, NGRP, 8, 16, 2), np.float32)  # b, g, fs, k, ri'
    for g in range(NGRP):
        for fs in range(8):
            fb = g * 8 + fs
            w16[:, g, fs, 0:8, 0] = wr[:, fb]
            w16[:, g, fs, 8:16, 0] = wi[:, fb]
            w16[:, g, fs, 0:8, 1] = -wi[:, fb]
            w16[:, g, fs, 8:16, 1] = wr[:, fb]
    out = []
    for c in range(NCORES):
        bank = np.zeros((P, NTW, 32), np.float32)
        for j in range(GPC):
            g = c * GPC + j
            coff = (j % 2) * 16
            for fs in range(8):
                bank[fs * 16 : (fs + 1) * 16, j::GPC, coff + fs * 2 : coff + fs * 2 + 2] = (
                    w16[:, g, fs].transpose(1, 0, 2)
                )
        out.append(bank.astype(ml_dtypes.bfloat16))
    return out


def _pack_x(inp, perm):
    """Per-core x tensors [128, GPC*N_FRAMES] bf16, per-chunk contiguous:
    partition p = fs*16 + ri*8 + ch; chunk q occupies cols
    [GPC*F0[q], GPC*F0[q+1]) as [g, frame-in-chunk]."""
    import ml_dtypes

    xb = np.asarray(inp, dtype=np.float32).astype(ml_dtypes.bfloat16)
    xs = xb[perm][:, :, :NBIN_DEV, :]  # (N, 2, 256, 8) bf16
    # (n, ri, c, g, fs, ch) -> [c, fs, ri, ch, g, n]
    arr = xs.reshape(N_FRAMES, 2, NCORES, GPC, 8, NUM_CH)
    arr = np.ascontiguousarray(arr.transpose(2, 4, 1, 5, 3, 0))
    arr = arr.reshape(NCORES, P, GPC, N_FRAMES)
    out = np.empty((NCORES, P, GPC * N_FRAMES), ml_dtypes.bfloat16)
    for q in range(NCH):
        blk = arr[:, :, :, F0[q] : F0[q + 1]]  # (NCORES, P, GPC, fch)
        out[:, :, GPC * F0[q] : GPC * F0[q + 1]] = blk.reshape(
            NCORES, P, GPC * CH_SIZES[q]
        )
    return out


def kernel(**inputs):
    global LAST_RESULTS
    from concourse.bass_utils import run_bass_kernel_spmd

    inp = np.ascontiguousarray(np.asarray(inputs["input"], dtype=np.float32))
    W = np.ascontiguousarray(np.asarray(inputs["W"], dtype=np.float32))
    bid = np.asarray(inputs["beam_id"]).astype(np.int64)

    perm = np.argsort(bid, kind="stable")
    counts = np.bincount(bid, minlength=NUM_BEAM)
    offs = np.concatenate([[0], np.cumsum(counts)]).astype(int)

    key = tuple(offs)
    if key not in _CACHE:
        _CACHE[key] = _build_program(offs)
    nc = _CACHE[key]

    wts = _pack_weights(W)
    xts = _pack_x(inp, perm)
    in_maps = [{"xt": xts[c], "wt": wts[c]} for c in range(NCORES)]

    res = run_bass_kernel_spmd(nc, in_maps, list(range(NCORES)), trace=TRACE)
    LAST_RESULTS = res

    # staging row (within half-chunk) = 64*half + 16*j + fs*2 + ri
    out_sorted = np.empty((N_FRAMES, 2, NUM_BIN), np.float32)
    for c in range(NCORES):
        ot = np.asarray(res.results[c]["out"]).astype(np.float32)  # (128, N/2)
        for q in range(NCH):
            hf = CH_SIZES[q] // 2
            blk = ot[:, OC[q] : OC[q + 1]]           # (128, hf)
            a = blk.reshape(2, GPC, 8, 2, hf)        # (half, j, fs, ri, f2)
            a = a.transpose(0, 4, 3, 1, 2)           # (half, f2, ri, j, fs)
            out_sorted[F0[q] : F0[q + 1], :, 32 * c : 32 * c + 32] = a.reshape(
                CH_SIZES[q], 2, 32
            )

    # bin 256 on host (keeps the device bin count divisible by 8 cores)
    xs = inp[:, :, NUM_BIN - 1, :]
    ws = W[bid][:, :, NUM_BIN - 1, :]
    xr, xi = xs[:, 0], xs[:, 1]
    wr, wi = ws[:, 0], ws[:, 1]

    out_full = np.empty((N_FRAMES, 2, NUM_BIN), np.float32)
    out_full[perm] = out_sorted
    out_full[:, 0, NUM_BIN - 1] = (xr * wr + xi * wi).sum(-1)
    out_full[:, 1, NUM_BIN - 1] = (xi * wr - xr * wi).sum(-1)
    return out_full.reshape(N_FRAMES, 2, NUM_BIN, 1)
